# revision 37
# baseline (speedup 1.0000x reference)
"""Trainium2 Bass kernel for nn_MultiHeadAttention_67018669687091.

Problem: MHA with B=2, S=2048, E=1024, H=16, D=64, causal, fp32.
The reference reshapes (B,S,E)->(B,H,S,D) WITHOUT transpose, so head h of
batch b is the contiguous 128-row x-block rows [h*128,(h+1)*128) viewed as a
(2048, 64) pseudo-sequence: position 16*s+j <- (row s, channel 64j+d).

Sharding: 8 cores; core c owns batch b=c//4 and head-quad qd=c%4 (4 heads).

v2 structure (per core):
 - QKV projection computed DIRECTLY in transposed form: out[e_out, s] =
   sum_e_in WqkvT[e_in, e_out] xT[e_in, s], with all 4 heads' s-ranges
   batched in the matmul free dim (N=512). No PE transposes for Q/K/V.
 - PSUM drains scatter (e_out-chunk, s) -> (d, permuted pseudo-col) into
   QTs/KTs/VTs; qkv bias folded in via per-partition-scalar add (DVE) /
   Identity-activation bias (ACT), alternating engines.
 - Per-head causal attention in the chunk-major permuted column domain
   (col = 128*kc + 8*jb + s8 <-> pos 16*(8*kc+s8)+jb). Causal mask applied
   ADDITIVELY (-1e9) to scores in PSUM before the exp, off the exp->PV edge.
 - Softmax denominator via augmented ones-row of V; reciprocal partition-
   spread via a DRAM round trip (deferred chain slices); the con write is a
   CONTIGUOUS DVE multiply (columns stay permuted; the host un-permutes).
 - Row-parallel output projection; sb=0,1 tiles emitted before the last
   normalization chain drains; paired [128,2,512] output DMAs split across
   the Sync and Scalar queues. Host sums the 4 partials per batch and adds
   bout.
"""
import numpy as np
from contextlib import ExitStack

import concourse.bass as bass
import concourse.bacc as bacc
import concourse.mybir as mybir
import concourse.tile as tile
from concourse.masks import make_identity
from concourse.bass_utils import run_bass_kernel_spmd

E = 1024
H = 16
D = 64
B = 2
S = 2048
HPC = 4          # heads per core
SL = HPC * 128   # x columns per core (512)

F32 = mybir.dt.float32
F32R = mybir.dt.float32r
BF16 = mybir.dt.bfloat16
EXP = mybir.ActivationFunctionType.Exp
IDENT = mybir.ActivationFunctionType.Identity


def _pieces(lo, hi, bank=512):
    """Split [lo, hi) at multiples of `bank` (PSUM bank boundaries)."""
    out = []
    while lo < hi:
        nxt = min(hi, (lo // bank + 1) * bank)
        out.append((lo, nxt))
        lo = nxt
    return out


def build_program(mm_dt=BF16):
    """One SPMD program; per-core data comes via in_maps."""
    assert mm_dt == BF16, "v3 kernel requires a 2-byte dtype (XBAR transpose)"
    nc = bacc.Bacc("TRN2", target_bir_lowering=False)
    DT = mm_dt
    QKVDT = DT if DT == BF16 else F32

    # Host-prepacked inputs (every load a clean 2D DMA):
    #   xq2[p, ec*512 + hl*128 + s] = x[b].T[ec*128+p, qd*512 + hl*128+s]
    #   wq[(t*8+ec)*128+p, c]       = Wqkv.T[ec*128+p, t*1024+c]
    #   bcol[p, t*8+c]              = bqkv[t*1024 + c*128 + p]
    #   woutq[p, hf*E+j]            = Wout.T[qd*256+hf*128+p, j]
    xq2 = nc.dram_tensor("xq2", [128, 8 * 512], DT, kind="ExternalInput").ap()
    wq = nc.dram_tensor("wq", [24 * 128, 1024], DT, kind="ExternalInput").ap()
    bcol = nc.dram_tensor("bcol", [128, 24], F32, kind="ExternalInput").ap()
    woutq = nc.dram_tensor("woutq", [128, 2 * E], DT, kind="ExternalInput").ap()
    maskd = nc.dram_tensor("maskp", [128, 128], F32, kind="ExternalInput").ap()
    # tile-major output: row block (sb*8+jc) holds the [128, 512] tile for
    # out channels jc*128.. and (permuted) positions sb*512.. — every DMA
    # write lands fully contiguous in DRAM; the host assembles/unpermutes.
    outT = nc.dram_tensor("partialT", [32 * 128, 512], BF16,
                          kind="ExternalOutput").ap()

    with tile.TileContext(nc) as tc, ExitStack() as ctx:
        singles = ctx.enter_context(tc.tile_pool(name="singles", bufs=1))
        wpool = ctx.enter_context(tc.tile_pool(name="wpool", bufs=2))
        ppool = ctx.enter_context(tc.tile_pool(name="ppool", bufs=4))
        cpool = ctx.enter_context(tc.tile_pool(name="cpool", bufs=1))
        stpool = ctx.enter_context(tc.tile_pool(name="stpool", bufs=4))
        small = ctx.enter_context(tc.tile_pool(name="small", bufs=3))
        dpool = ctx.enter_context(tc.tile_pool(name="dpool", bufs=2, space="DRAM"))

        # Additive causal mask (0 / -1e9, f32) for the diagonal 128-chunk,
        # host-computed for the permuted key/query order.
        mask01 = singles.tile([128, 128], F32, tag="mask01")

        # Persistent transposed-domain tensors. Head hl occupies columns
        # [hl*2048, (hl+1)*2048); rows 0:64 = d, rows 64:128 zero-padded so
        # score matmuls run K=128 (K=64 serializes LDWEIGHTS). VTs row 64 is
        # the all-ones denominator row; VTs is padded to 80 partitions
        # (XBAR_TILE_SRC_ROWS=16) so ONE SBUF->SBUF DMA transpose produces
        # Vcs[p, chunk, r] = VTs[r, chunk*128+p] — per-chunk V with the ones
        # column riding through at r=64 (cols 65:80 are unread junk).
        QTs = singles.tile([128, HPC * S], DT, tag="QTs")
        KTs = singles.tile([128, HPC * S], DT, tag="KTs")
        VTs = singles.tile([80, HPC * S], QKVDT, tag="VTs")
        Vcs = singles.tile([128, 64, 80], DT, tag="Vcs")

        conA = cpool.tile([128, S], DT, tag="conA")
        conB = cpool.tile([128, S], DT, tag="conB")

        # Startup loads. x chunks + bias + wout issue from the Scalar HWDGE
        # queue, weights (V tensor first) from Sync — the two issuers run in
        # parallel; the first matmul needs only xe[0] + the first w chunk.
        # first two x chunks load individually (small first transfer -> the
        # first matmul starts sooner); the rest in pairs (fewer issues)
        xaps = {}
        for ec in range(2):
            xe = singles.tile([128, 512], DT, tag=f"xe{ec}", name=f"xe{ec}")
            nc.scalar.dma_start(out=xe, in_=xq2[:, ec * 512:(ec + 1) * 512])
            xaps[ec] = xe
        for u in range(1, 4):
            xp = singles.tile([128, 2, 512], DT, tag=f"xp{u}", name=f"xp{u}")
            nc.scalar.dma_start(
                out=xp,
                in_=xq2[:, u * 1024:(u + 1) * 1024].rearrange(
                    "p (two s) -> p two s", two=2))
            xaps[2 * u] = xp[:, 0, :]
            xaps[2 * u + 1] = xp[:, 1, :]
        bcol_sb = singles.tile([128, 24], F32, tag="bcol")
        nc.scalar.dma_start(out=bcol_sb, in_=bcol)
        wout_sb = singles.tile([128, 2, E], DT, tag="wout")

        # zero pads / ones row (needed only from attention onward)
        nc.gpsimd.memset(QTs[64:128, :], 0.0)
        nc.gpsimd.memset(KTs[64:128, :], 0.0)
        nc.gpsimd.memset(VTs[64:65, :], 1.0)
        ones1 = singles.tile([1, 64], DT, tag="ones1")
        nc.gpsimd.memset(ones1, 1.0)

        DEST = {0: QTs, 1: KTs, 2: VTs}

        def emit_drain(t, c, ps):
            # ps[64*jh+d', hl*128 + 8*kc + s8] -> DEST[t][d', permuted col]
            # for j = 2c+jh; bias bqkv[t*1024 + c*128 + p] folded in.
            dest = DEST[t]
            dst5 = dest.rearrange(
                "d (hl kc jb s8) -> d hl kc jb s8", hl=HPC, kc=16, jb=16)
            for jh in range(2):
                src = ps[64 * jh:64 * jh + 64, :].rearrange(
                    "d (hl kc s8) -> d hl kc s8", hl=HPC, kc=16)
                dst = dst5[0:64, :, :, 2 * c + jh, :]
                bias_ap = bcol_sb[64 * jh:64 * jh + 64, t * 8 + c:t * 8 + c + 1]
                if jh == 0:
                    nc.vector.tensor_scalar_add(out=dst, in0=src, scalar1=bias_ap)
                else:
                    nc.scalar.activation(dst, src, IDENT, bias=bias_ap)

        # ---- QKV projection, transposed form. t order (2,0,1): V first so
        # its drains complete long before the Vc transpose needs them.
        # Each tensor runs as 4 quarter-phases of 2 accumulation chains so
        # drains pipeline mid-phase instead of tailing into attention.
        proj_ctx = ExitStack()
        pp = proj_ctx.enter_context(tc.tile_pool(name="pp", bufs=8, space="PSUM"))
        for ti, t in enumerate((2, 0, 1)):
            waps = {}
            for ec in range(2):
                wtc = wpool.tile([128, 1024], DT, tag=f"wt{ec}",
                                 name=f"wt{t}_{ec}")
                r = (t * 8 + ec) * 128
                nc.sync.dma_start(out=wtc, in_=wq[r:r + 128, :])
                waps[ec] = wtc
            for u in range(1, 4):
                wtc = wpool.tile([128, 2, 1024], DT, tag=f"wp{u}",
                                 name=f"wp{t}_{u}")
                r = (t * 8 + 2 * u) * 128
                nc.sync.dma_start(
                    out=wtc,
                    in_=wq[r:r + 256, :].rearrange("(two p) c -> p two c", two=2))
                waps[2 * u] = wtc[:, 0, :]
                waps[2 * u + 1] = wtc[:, 1, :]
            if ti == 0:
                nc.sync.dma_start(out=mask01, in_=maskd)
            for q in range(4):
                cs = (2 * q, 2 * q + 1)
                pss = {c: pp.tile([128, 512], F32, tag="pp", name=f"pp{t}_{c}")
                       for c in cs}
                for ec in range(8):
                    for c in cs:
                        nc.tensor.matmul(
                            pss[c],
                            lhsT=waps[ec][:, c * 128:(c + 1) * 128],
                            rhs=xaps[ec],
                            start=(ec == 0), stop=(ec == 7),
                        )
                for c in cs:
                    emit_drain(t, c, pss[c])
            if ti == 0:
                # Vc via ONE XBAR DMA transpose (scalar HWDGE queue, idle
                # here); completes mid-projection, long before the first PV.
                nc.scalar.dma_start_transpose(out=Vcs, in_=VTs)
        nc.scalar.dma_start(
            out=wout_sb, in_=woutq.rearrange("p (hf j) -> p hf j", hf=2))
        proj_ctx.close()

        attn_ctx = ExitStack()
        ops = attn_ctx.enter_context(tc.tile_pool(name="ops", bufs=1, space="PSUM"))
        qh0_ctx = ExitStack()
        spool = {}
        spool['p'] = qh0_ctx.enter_context(
            tc.tile_pool(name="spsA", bufs=3, space="PSUM"))

        def emit_attention_bf16(hl, qh, outp, filler=None):
            # Key chunks whose query lengths sum to 1024 share one St tile
            # and ONE exp (the causal staircase pairs up exactly).
            q0 = hl * S
            items = []
            for kc in range(8 * (qh + 1)):
                qstart = max(kc * 128, qh * 1024)
                items.append((kc, qstart, (qh + 1) * 1024 - qstart))
            full = [[it] for it in items if it[2] >= 1024]
            rest = sorted((it for it in items if it[2] < 1024),
                          key=lambda it: -it[2])
            groups = list(full)
            i, j = 0, len(rest) - 1
            while i <= j:
                if i < j and rest[i][2] + rest[j][2] <= 1024:
                    groups.append([rest[i], rest[j]])
                    i, j = i + 1, j - 1
                else:
                    groups.append([rest[i]])
                    i += 1
            groups.sort(key=lambda g: min(it[0] for it in g))
            # per-PSUM-bank last writer under the actual emission order
            bank_last = {}
            for g in groups:
                for (kc, qstart, qlen) in g:
                    rel = qstart - qh * 1024
                    for (a, b) in _pieces(rel, rel + qlen):
                        bank_last[a // 512] = kc
            for g in groups:
                if filler is not None:
                    filler()
                St = spool['p'].tile([128, 1024], F32, tag="S", space="PSUM",
                                     name=f"St{hl}_{qh}_{g[0][0]}")
                off, offs = 0, []
                for (kc, qstart, qlen) in g:
                    for (a, b) in _pieces(off, off + qlen):
                        nc.tensor.matmul(
                            St[:, a:b],
                            lhsT=KTs[:, q0 + kc * 128: q0 + (kc + 1) * 128],
                            rhs=QTs[:, q0 + qstart + a - off: q0 + qstart + b - off],
                            start=True, stop=True,
                        )
                    if kc * 128 == qstart:
                        # additive -1e9 on the diagonal chunk, pre-exp
                        nc.vector.tensor_add(
                            St[:, off:off + 128], St[:, off:off + 128], mask01)
                    offs.append(off)
                    off += qlen
                P = ppool.tile([128, 1024], DT, tag="P",
                               name=f"P{hl}_{qh}_{g[0][0]}")
                nc.scalar.activation(P[:, 0:off], St[:, 0:off], EXP, scale=0.125)
                for (kc, qstart, qlen), o in zip(g, offs):
                    rel = qstart - qh * 1024
                    for (a, b) in _pieces(rel, rel + qlen):
                        nc.tensor.matmul(
                            outp[a // 512][:, a % 512:a % 512 + b - a],
                            lhsT=Vcs[:, hl * 16 + kc, 0:65],
                            rhs=P[:, o + a - rel: o + b - rel],
                            start=(kc == 0), stop=(kc == bank_last[a // 512]),
                        )

        # ---- softmax-denominator normalization chains, emitted in slices
        # across later flush points so no in-order engine queue waits on a
        # DMA round-trip hop in flight.
        chains = []

        def flush_chains():
            for ch in list(chains):
                ch.pop(0)()
                if not ch:
                    chains.remove(ch)

        def emit_attention(hl, qh, filler=None):
            con = conA if hl < 2 else conB
            r0 = 64 * (hl % 2)
            outpt = ops.tile([96, 1024], F32, tag="outp", space="PSUM",
                             name=f"outp{hl}_{qh}")
            outp = [outpt[0:65, 0:512], outpt[0:65, 512:1024]]
            emit_attention_bf16(hl, qh, outp, filler)
            box = {}
            if hl == 3 and qh == 1:
                # Last chain: all-on-chip partition spread (no DRAM hops to
                # hide at the very end). The den row is copied to a base-0
                # SBUF tile, DVE 32x32 block transposes put it on
                # partitions, reciprocal runs 32-wide, a second transpose
                # restores a row, and a K=1 PE outer product broadcasts it
                # into PSUM for the con multiply.
                den32 = small.tile([32, 1024], F32, tag="den32", name="den32")
                nc.vector.tensor_copy(den32[0:1, :], outpt[64:65, :])
            stg = small.tile([65, 1024], F32, tag="stg", name=f"stg{hl}_{qh}")
            nc.vector.tensor_copy(stg, outpt[0:65, :])
            if hl == 3 and qh == 1:
                def s2m():
                    tsp = small.tile([32, 1024], F32, tag="tsp", name="tsp")
                    nc.vector.transpose(tsp, den32)
                    rsp = small.tile([32, 1024], F32, tag="rsp", name="rsp")
                    nc.vector.reciprocal(
                        rsp.rearrange("p (b c) -> p b c", c=32)[:, :, 0:1],
                        tsp.rearrange("p (b c) -> p b c", c=32)[:, :, 0:1])
                    r16 = small.tile([32, 1024], BF16, tag="r16", name="r16")
                    nc.vector.tensor_copy(
                        r16.rearrange("p (b c) -> p b c", c=32)[:, :, 0:1],
                        rsp.rearrange("p (b c) -> p b c", c=32)[:, :, 0:1])
                    rT = small.tile([32, 1024], BF16, tag="rT", name="rT")
                    nc.vector.transpose(rT, r16)
                    rec64p = recpool.tile([64, 1024], F32, tag="recp",
                                          space="PSUM", name="recp")
                    for h in range(2):
                        nc.tensor.matmul(
                            rec64p[:, h * 512:(h + 1) * 512],
                            lhsT=ones1,
                            rhs=rT[0:1, h * 512:(h + 1) * 512],
                            start=True, stop=True)
                    box["rec64"] = rec64p

                def s3m():
                    nc.vector.tensor_mul(
                        con[r0:r0 + 64, qh * 1024:(qh + 1) * 1024],
                        stg[0:64, :], box["rec64"])

                # dummy first slice: the real work must land at the final
                # flush, after the end-phase PSUM pools exist
                chains.append([lambda: None, lambda: (s2m(), s3m())])
                return
            # denominator row, partition-spread via DRAM (a (1,1024)
            # single-lane DVE reciprocal costs 6.5us; spread across 128
            # partitions it is ~60ns).
            d_dram = dpool.tile([1, 1024], F32, tag="d_dram", name=f"dd{hl}_{qh}")
            nc.sync.dma_start(out=d_dram, in_=stg[64:65, :])
            spread = small.tile([128, 8], F32, tag="spread", name=f"sp{hl}_{qh}")
            nc.sync.dma_start(
                out=spread,
                in_=d_dram.rearrange("a (p i) -> p a i", p=128)[:, 0, :],
            )

            def s2():
                rspread = small.tile([128, 8], F32, tag="rspread",
                                     name=f"rs{hl}_{qh}")
                nc.vector.reciprocal(rspread, spread)
                # bf16 for the respread round trip: halves the broadcast
                # read (the chain's longest hop); |rec| rel err ~2^-9 is
                # well inside the output tolerance.
                rs16 = small.tile([128, 8], BF16, tag="rs16",
                                  name=f"rs16{hl}_{qh}")
                nc.vector.tensor_copy(rs16, rspread)
                r_dram = dpool.tile([1, 1024], BF16, tag="r_dram",
                                    name=f"rd{hl}_{qh}")
                nc.sync.dma_start(
                    out=r_dram.rearrange("a (p i) -> p a i", p=128)[:, 0, :],
                    in_=rs16,
                )
                rec64 = small.tile([64, 1024], BF16, tag="rec64",
                                   name=f"r64{hl}_{qh}")
                rec_bcast = bass.AP(
                    tensor=r_dram.tensor, offset=r_dram.offset,
                    ap=[[0, 64]] + [list(d) for d in r_dram.ap[1:]],
                )
                nc.sync.dma_start(out=rec64, in_=rec_bcast)
                box["rec64"] = rec64

            def s3():
                # columns stay chunk-major permuted end-to-end: contiguous
                # multiply; the host un-permutes the final output columns.
                nc.vector.tensor_mul(
                    con[r0:r0 + 64, qh * 1024:(qh + 1) * 1024],
                    stg[0:64, :], box["rec64"])

            chains.append([s2, s3])

        # ---- output projection helpers: partialT[j, s] = woutT_s.T @
        # [conA; conB]. The 16 sb=0,1 tiles (which need only the qh=0
        # halves of con) are interleaved one-per-group into the last two
        # attention blocks: their matmuls have no exp dependence, so they
        # absorb the ACT engine's per-group deficit that otherwise stalls
        # the score pipeline, and they shrink the end phase.
        def mmA(ps, sb, jc, stop=False):
            nc.tensor.matmul(
                ps, lhsT=wout_sb[:, 0, jc * 128:(jc + 1) * 128],
                rhs=conA[:, sb * 512:(sb + 1) * 512], start=True, stop=stop)

        def mmB(ps, sb, jc):
            nc.tensor.matmul(
                ps, lhsT=wout_sb[:, 1, jc * 128:(jc + 1) * 128],
                rhs=conB[:, sb * 512:(sb + 1) * 512], start=False, stop=True)

        def finish_pair(sb, m, stage_tiles, engines):
            sts = stpool.tile([128, 2, 512], BF16, tag="st")
            for u in range(2):
                if engines[u] == 's':
                    nc.scalar.copy(sts[:, u, :], stage_tiles[u])
                elif engines[u] == 'g':
                    nc.gpsimd.tensor_copy(sts[:, u, :], stage_tiles[u])
                else:
                    nc.vector.tensor_copy(sts[:, u, :], stage_tiles[u])
            r = (sb * 8 + 2 * m) * 128
            dst = outT[r:r + 256, :].rearrange("(two p) s -> p two s", two=2)
            return sts, dst

        # interleaved sb=0,1 out-proj tiles, one per attention group of the
        # last two blocks; staged on Vector, written on Sync
        sb01_tiles = [(sb, m, u) for sb in (0, 1) for m in range(4)
                      for u in range(2)]
        fstate = {'i': 0, 'sts': None}

        def filler():
            if fstate['i'] >= len(sb01_tiles):
                return
            sb, m, u = sb01_tiles[fstate['i']]
            if u == 0:
                fstate['sts'] = stpool.tile([128, 2, 512], BF16, tag="st",
                                            name=f"st{fstate['i']}")
            ps = opearly.tile([128, 512], F32, tag="ope",
                              name=f"ope{fstate['i']}")
            mmA(ps, sb, 2 * m + u)
            mmB(ps, sb, 2 * m + u)
            nc.vector.tensor_copy(fstate['sts'][:, u, :], ps)
            if u == 1:
                r = (sb * 8 + 2 * m) * 128
                dst = outT[r:r + 256, :].rearrange(
                    "(two p) s -> p two s", two=2)
                nc.sync.dma_start(out=dst, in_=fstate['sts'])
            fstate['i'] += 1

        for hl in range(HPC):
            emit_attention(hl, 0)
            flush_chains()
        for hl in range(HPC):
            if hl == 3:
                # swap to a shallower score pool to free 2 PSUM banks for
                # the interleaved out-proj tiles (the filler's PE work
                # keeps the exp pipeline fed despite the shallower ring)
                qh0_ctx.close()
                spool['p'] = attn_ctx.enter_context(
                    tc.tile_pool(name="spsB", bufs=2, space="PSUM"))
                opearly = attn_ctx.enter_context(
                    tc.tile_pool(name="opearly", bufs=2, space="PSUM"))
            emit_attention(hl, 1, filler if hl == 3 else None)
            flush_chains()
        while fstate['i'] < len(sb01_tiles):
            filler()
        attn_ctx.close()
        oproj = ctx.enter_context(tc.tile_pool(name="oproj", bufs=6, space="PSUM"))
        recpool = ctx.enter_context(
            tc.tile_pool(name="recpool", bufs=1, space="PSUM"))

        # the last chain's s3 is pure Vector work whose DMA inputs are
        # already in flight — emit it first so it rides an empty queue
        while chains:
            flush_chains()
        pss2 = []
        for m in range(3):
            for u in range(2):
                ps = oproj.tile([128, 512], F32, tag="op")
                mmA(ps, 2, 2 * m + u)
                pss2.append(ps)
        for m in range(3):
            for u in range(2):
                mmB(pss2[2 * m + u], 2, 2 * m + u)
            sts, dst = finish_pair(2, m, pss2[2 * m:2 * m + 2], 'sv')
            nc.sync.dma_start(out=dst, in_=sts)
        for m in range(3, 4):
            pss = []
            for u in range(2):
                ps = oproj.tile([128, 512], F32, tag="op")
                mmA(ps, 2, 2 * m + u)
                mmB(ps, 2, 2 * m + u)
                pss.append(ps)
            sts, dst = finish_pair(2, m, pss, 'sv')
            nc.sync.dma_start(out=dst, in_=sts)
        for m in range(4):
            pss = []
            for u in range(2):
                ps = oproj.tile([128, 512], F32, tag="op")
                mmA(ps, 3, 2 * m + u)
                mmB(ps, 3, 2 * m + u)
                pss.append(ps)
            sts, dst = finish_pair(3, m, pss, 'sv')
            nc.sync.dma_start(out=dst, in_=sts)
    nc.compile()
    return nc


def make_in_maps(x, Wqkv, bqkv, Wout, mm_dt=BF16):
    np_dt = mybir.dt.np(mm_dt)
    x = np.asarray(x, np.float32)
    xT = np.ascontiguousarray(x.transpose(0, 2, 1)).astype(np_dt)  # (2,1024,2048)
    WqkvT = np.asarray(Wqkv, np.float32).T.astype(np_dt)
    WoutT = np.asarray(Wout, np.float32).T.astype(np_dt)
    # wq[(t*8+ec)*128+p, c] = WqkvT[ec*128+p, t*1024+c]
    wqh = np.ascontiguousarray(
        WqkvT.reshape(8, 128, 3, 1024).transpose(2, 0, 1, 3).reshape(3072, 1024))
    # bcol[p, t*8+c] = bqkv[t*1024 + c*128 + p]
    bcolh = np.ascontiguousarray(
        np.asarray(bqkv, np.float32).reshape(3, 8, 128).transpose(2, 0, 1)
        .reshape(128, 24))
    # additive mask for the diagonal 128-chunk: key partitions AND query
    # columns both in within-chunk scatter order (idx=8*jb+s8 -> 16*s8+jb).
    p = np.arange(128)
    pos = 16 * (p % 8) + p // 8
    allowed = pos[:, None] <= pos[None, :]
    maskp = np.where(allowed, 0.0, -1e9).astype(np.float32)
    in_maps = []
    for c in range(8):
        b, qd = divmod(c, 4)
        xc = xT[b][:, qd * SL:(qd + 1) * SL]  # [1024, 512]
        # xq2[p, ec*512 + hl*128 + s] = xc[ec*128+p, hl*128+s]
        xq2h = np.ascontiguousarray(
            xc.reshape(8, 128, 512).transpose(1, 0, 2).reshape(128, 4096))
        wo = WoutT[qd * 256:(qd + 1) * 256, :]  # [256, 1024]
        woq = np.ascontiguousarray(
            wo.reshape(2, 128, E).transpose(1, 0, 2).reshape(128, 2 * E))
        in_maps.append({
            "xq2": xq2h,
            "wq": wqh,
            "bcol": bcolh,
            "woutq": woq,
            "maskp": maskp,
        })
    return in_maps


_NC_CACHE = {}


def get_program(mm_dt=BF16):
    key = ("v2", str(mm_dt))
    if key not in _NC_CACHE:
        _NC_CACHE[key] = build_program(mm_dt)
    return _NC_CACHE[key]


def assemble(results, bout):
    bout = np.asarray(bout, np.float32)
    out = np.zeros((B, S, E), np.float32)
    for c in range(8):
        b = c // 4
        # tile-major [sb, jc, j, permuted scol] -> un-permute columns
        # (stored c4*128 + jb*8 + s8 <-> true c4*128 + 16*s8 + jb) ->
        # [S, E]; partial sum over the 4 head-quads.
        pt = results[c]["partialT"].reshape(4, 8, 128, 4, 16, 8)
        pt = pt.astype(np.float32)
        out[b] += pt.transpose(0, 3, 5, 4, 1, 2).reshape(S, E)
    out += bout
    return out


def kernel(x, Wqkv, bqkv, Wout, bout, mm_dt=BF16, trace=False):
    nc = get_program(mm_dt)
    in_maps = make_in_maps(x, Wqkv, bqkv, Wout, mm_dt)
    res = run_bass_kernel_spmd(nc, in_maps, list(range(8)), trace=trace)
    out = assemble(res.results, bout)
    if trace:
        kernel.last_result = res
    return out


# revision 38
# speedup vs baseline: 1.1312x; 1.1312x over previous
"""Trainium2 Bass kernel for nn_MultiHeadAttention_67018669687091.

Problem: MHA with B=2, S=2048, E=1024, H=16, D=64, causal, fp32.
The reference reshapes (B,S,E)->(B,H,S,D) WITHOUT transpose, so head h of
batch b is the contiguous 128-row x-block rows [h*128,(h+1)*128) viewed as a
(2048, 64) pseudo-sequence: position 16*s+j <- (row s, channel 64j+d).

Sharding: 8 cores; core c owns batch b=c//4 and head-quad qd=c%4 (4 heads).

v2 structure (per core):
 - QKV projection computed DIRECTLY in transposed form: out[e_out, s] =
   sum_e_in WqkvT[e_in, e_out] xT[e_in, s], with all 4 heads' s-ranges
   batched in the matmul free dim (N=512). No PE transposes for Q/K/V.
 - PSUM drains scatter (e_out-chunk, s) -> (d, permuted pseudo-col) into
   QTs/KTs/VTs; qkv bias folded in via per-partition-scalar add (DVE) /
   Identity-activation bias (ACT), alternating engines.
 - Per-head causal attention in the chunk-major permuted column domain
   (col = 128*kc + 8*jb + s8 <-> pos 16*(8*kc+s8)+jb). Causal mask applied
   ADDITIVELY (-1e9) to scores in PSUM before the exp, off the exp->PV edge.
 - Softmax denominator via augmented ones-row of V; reciprocal partition-
   spread via a DRAM round trip (deferred chain slices); the con write is a
   CONTIGUOUS DVE multiply (columns stay permuted; the host un-permutes).
 - Row-parallel output projection; sb=0,1 tiles emitted before the last
   normalization chain drains; paired [128,2,512] output DMAs split across
   the Sync and Scalar queues. Host sums the 4 partials per batch and adds
   bout.
"""
import numpy as np
from contextlib import ExitStack

import concourse.bass as bass
import concourse.bacc as bacc
import concourse.mybir as mybir
import concourse.tile as tile
from concourse.masks import make_identity
from concourse.bass_utils import run_bass_kernel_spmd

E = 1024
H = 16
D = 64
B = 2
S = 2048
HPC = 4          # heads per core
SL = HPC * 128   # x columns per core (512)

F32 = mybir.dt.float32
F32R = mybir.dt.float32r
BF16 = mybir.dt.bfloat16
EXP = mybir.ActivationFunctionType.Exp
IDENT = mybir.ActivationFunctionType.Identity


def _pieces(lo, hi, bank=512):
    """Split [lo, hi) at multiples of `bank` (PSUM bank boundaries)."""
    out = []
    while lo < hi:
        nxt = min(hi, (lo // bank + 1) * bank)
        out.append((lo, nxt))
        lo = nxt
    return out


def build_program(mm_dt=BF16):
    """One SPMD program; per-core data comes via in_maps."""
    assert mm_dt == BF16, "v3 kernel requires a 2-byte dtype (XBAR transpose)"
    nc = bacc.Bacc("TRN2", target_bir_lowering=False)
    DT = mm_dt
    QKVDT = DT if DT == BF16 else F32

    # Host-prepacked inputs (every load a clean 2D DMA):
    #   xq2[p, ec*512 + hl*128 + s] = x[b].T[ec*128+p, qd*512 + hl*128+s]
    #   wq[(t*8+ec)*128+p, c]       = Wqkv.T[ec*128+p, t*1024+c]
    #   bcol[p, t*8+c]              = bqkv[t*1024 + c*128 + p]
    #   woutq[p, hf*E+j]            = Wout.T[qd*256+hf*128+p, j]
    xq2 = nc.dram_tensor("xq2", [128, 8 * 512], DT, kind="ExternalInput").ap()
    wq = nc.dram_tensor("wq", [24 * 128, 1024], DT, kind="ExternalInput").ap()
    bcol = nc.dram_tensor("bcol", [128, 24], F32, kind="ExternalInput").ap()
    woutq = nc.dram_tensor("woutq", [128, 2 * E], DT, kind="ExternalInput").ap()
    maskd = nc.dram_tensor("maskp", [128, 128], F32, kind="ExternalInput").ap()
    # tile-major output: row block (sb*8+jc) holds the [128, 512] tile for
    # out channels jc*128.. and (permuted) positions sb*512.. — every DMA
    # write lands fully contiguous in DRAM; the host assembles/unpermutes.
    outT = nc.dram_tensor("partialT", [32 * 128, 512], BF16,
                          kind="ExternalOutput").ap()

    with tile.TileContext(nc) as tc, ExitStack() as ctx:
        singles = ctx.enter_context(tc.tile_pool(name="singles", bufs=1))
        wpool = ctx.enter_context(tc.tile_pool(name="wpool", bufs=2))
        ppool = ctx.enter_context(tc.tile_pool(name="ppool", bufs=4))
        cpool = ctx.enter_context(tc.tile_pool(name="cpool", bufs=1))
        stpool = ctx.enter_context(tc.tile_pool(name="stpool", bufs=4))
        small = ctx.enter_context(tc.tile_pool(name="small", bufs=3))
        dpool = ctx.enter_context(tc.tile_pool(name="dpool", bufs=2, space="DRAM"))

        # Additive causal mask (0 / -1e9, f32) for the diagonal 128-chunk,
        # host-computed for the permuted key/query order.
        mask01 = singles.tile([128, 128], F32, tag="mask01")

        # Persistent transposed-domain tensors. Head hl occupies columns
        # [hl*2048, (hl+1)*2048); rows 0:64 = d, rows 64:128 zero-padded so
        # score matmuls run K=128 (K=64 serializes LDWEIGHTS). VTs row 64 is
        # the all-ones denominator row; VTs is padded to 80 partitions
        # (XBAR_TILE_SRC_ROWS=16) so ONE SBUF->SBUF DMA transpose produces
        # Vcs[p, chunk, r] = VTs[r, chunk*128+p] — per-chunk V with the ones
        # column riding through at r=64 (cols 65:80 are unread junk).
        QTs = singles.tile([128, HPC * S], DT, tag="QTs")
        KTs = singles.tile([128, HPC * S], DT, tag="KTs")
        VTs = singles.tile([80, HPC * S], QKVDT, tag="VTs")
        Vcs = singles.tile([128, 64, 80], DT, tag="Vcs")

        conA = cpool.tile([128, S], DT, tag="conA")
        conB = cpool.tile([128, S], DT, tag="conB")

        # Startup loads. x chunks + bias + wout issue from the Scalar HWDGE
        # queue, weights (V tensor first) from Sync — the two issuers run in
        # parallel; the first matmul needs only xe[0] + the first w chunk.
        # first two x chunks load individually (small first transfer -> the
        # first matmul starts sooner); the rest in pairs (fewer issues)
        xaps = {}
        for ec in range(2):
            xe = singles.tile([128, 512], DT, tag=f"xe{ec}", name=f"xe{ec}")
            nc.scalar.dma_start(out=xe, in_=xq2[:, ec * 512:(ec + 1) * 512])
            xaps[ec] = xe
        for u in range(1, 4):
            xp = singles.tile([128, 2, 512], DT, tag=f"xp{u}", name=f"xp{u}")
            nc.scalar.dma_start(
                out=xp,
                in_=xq2[:, u * 1024:(u + 1) * 1024].rearrange(
                    "p (two s) -> p two s", two=2))
            xaps[2 * u] = xp[:, 0, :]
            xaps[2 * u + 1] = xp[:, 1, :]
        bcol_sb = singles.tile([128, 24], F32, tag="bcol")
        nc.scalar.dma_start(out=bcol_sb, in_=bcol)
        wout_sb = singles.tile([128, 2, E], DT, tag="wout")

        # zero pads / ones row (needed only from attention onward)
        nc.gpsimd.memset(QTs[64:128, :], 0.0)
        nc.gpsimd.memset(KTs[64:128, :], 0.0)
        nc.gpsimd.memset(VTs[64:65, :], 1.0)
        ones1 = singles.tile([1, 64], DT, tag="ones1")
        nc.gpsimd.memset(ones1, 1.0)

        DEST = {0: QTs, 1: KTs, 2: VTs}

        def emit_drain(t, c, ps):
            # ps[64*jh+d', hl*128 + 8*kc + s8] -> DEST[t][d', permuted col]
            # for j = 2c+jh; bias bqkv[t*1024 + c*128 + p] folded in.
            dest = DEST[t]
            dst5 = dest.rearrange(
                "d (hl kc jb s8) -> d hl kc jb s8", hl=HPC, kc=16, jb=16)
            for jh in range(2):
                src = ps[64 * jh:64 * jh + 64, :].rearrange(
                    "d (hl kc s8) -> d hl kc s8", hl=HPC, kc=16)
                dst = dst5[0:64, :, :, 2 * c + jh, :]
                bias_ap = bcol_sb[64 * jh:64 * jh + 64, t * 8 + c:t * 8 + c + 1]
                if jh == 0:
                    nc.vector.tensor_scalar_add(out=dst, in0=src, scalar1=bias_ap)
                else:
                    nc.scalar.activation(dst, src, IDENT, bias=bias_ap)

        # ---- QKV projection, transposed form. t order (2,0,1): V first so
        # its drains complete long before the Vc transpose needs them.
        # Each tensor runs as 4 quarter-phases of 2 accumulation chains so
        # drains pipeline mid-phase instead of tailing into attention.
        proj_ctx = ExitStack()
        pp = proj_ctx.enter_context(tc.tile_pool(name="pp", bufs=8, space="PSUM"))
        for ti, t in enumerate((2, 0, 1)):
            waps = {}
            for ec in range(2):
                wtc = wpool.tile([128, 1024], DT, tag=f"wt{ec}",
                                 name=f"wt{t}_{ec}")
                r = (t * 8 + ec) * 128
                nc.sync.dma_start(out=wtc, in_=wq[r:r + 128, :])
                waps[ec] = wtc
            for u in range(1, 4):
                wtc = wpool.tile([128, 2, 1024], DT, tag=f"wp{u}",
                                 name=f"wp{t}_{u}")
                r = (t * 8 + 2 * u) * 128
                nc.sync.dma_start(
                    out=wtc,
                    in_=wq[r:r + 256, :].rearrange("(two p) c -> p two c", two=2))
                waps[2 * u] = wtc[:, 0, :]
                waps[2 * u + 1] = wtc[:, 1, :]
            if ti == 0:
                nc.sync.dma_start(out=mask01, in_=maskd)
            for q in range(4):
                cs = (2 * q, 2 * q + 1)
                pss = {c: pp.tile([128, 512], F32, tag="pp", name=f"pp{t}_{c}")
                       for c in cs}
                for ec in range(8):
                    for c in cs:
                        nc.tensor.matmul(
                            pss[c],
                            lhsT=waps[ec][:, c * 128:(c + 1) * 128],
                            rhs=xaps[ec],
                            start=(ec == 0), stop=(ec == 7),
                        )
                for c in cs:
                    emit_drain(t, c, pss[c])
            if ti == 0:
                # Vc via ONE XBAR DMA transpose (scalar HWDGE queue, idle
                # here); completes mid-projection, long before the first PV.
                nc.scalar.dma_start_transpose(out=Vcs, in_=VTs)
        nc.scalar.dma_start(
            out=wout_sb, in_=woutq.rearrange("p (hf j) -> p hf j", hf=2))
        proj_ctx.close()

        attn_ctx = ExitStack()
        ops = attn_ctx.enter_context(tc.tile_pool(name="ops", bufs=1, space="PSUM"))
        qh0_ctx = ExitStack()
        spool = {}
        spool['p'] = qh0_ctx.enter_context(
            tc.tile_pool(name="spsA", bufs=3, space="PSUM"))

        def emit_attention_bf16(hl, qh, outp, filler=None):
            # Key chunks whose query lengths sum to 1024 share one St tile
            # and ONE exp (the causal staircase pairs up exactly).
            q0 = hl * S
            items = []
            for kc in range(8 * (qh + 1)):
                qstart = max(kc * 128, qh * 1024)
                items.append((kc, qstart, (qh + 1) * 1024 - qstart))
            full = [[it] for it in items if it[2] >= 1024]
            rest = sorted((it for it in items if it[2] < 1024),
                          key=lambda it: -it[2])
            groups = list(full)
            i, j = 0, len(rest) - 1
            while i <= j:
                if i < j and rest[i][2] + rest[j][2] <= 1024:
                    groups.append([rest[i], rest[j]])
                    i, j = i + 1, j - 1
                else:
                    groups.append([rest[i]])
                    i += 1
            groups.sort(key=lambda g: min(it[0] for it in g))
            # per-PSUM-bank last writer under the actual emission order
            bank_last = {}
            for g in groups:
                for (kc, qstart, qlen) in g:
                    rel = qstart - qh * 1024
                    for (a, b) in _pieces(rel, rel + qlen):
                        bank_last[a // 512] = kc
            for g in groups:
                if filler is not None:
                    filler()
                St = spool['p'].tile([128, 1024], F32, tag="S", space="PSUM",
                                     name=f"St{hl}_{qh}_{g[0][0]}")
                off, offs = 0, []
                for (kc, qstart, qlen) in g:
                    for (a, b) in _pieces(off, off + qlen):
                        nc.tensor.matmul(
                            St[:, a:b],
                            lhsT=KTs[:, q0 + kc * 128: q0 + (kc + 1) * 128],
                            rhs=QTs[:, q0 + qstart + a - off: q0 + qstart + b - off],
                            start=True, stop=True,
                        )
                    if kc * 128 == qstart:
                        # additive -1e9 on the diagonal chunk, pre-exp
                        nc.vector.tensor_add(
                            St[:, off:off + 128], St[:, off:off + 128], mask01)
                    offs.append(off)
                    off += qlen
                P = ppool.tile([128, 1024], DT, tag="P",
                               name=f"P{hl}_{qh}_{g[0][0]}")
                nc.scalar.activation(P[:, 0:off], St[:, 0:off], EXP, scale=0.125)
                for (kc, qstart, qlen), o in zip(g, offs):
                    rel = qstart - qh * 1024
                    for (a, b) in _pieces(rel, rel + qlen):
                        nc.tensor.matmul(
                            outp[a // 512][:, a % 512:a % 512 + b - a],
                            lhsT=Vcs[:, hl * 16 + kc, 0:65],
                            rhs=P[:, o + a - rel: o + b - rel],
                            start=(kc == 0), stop=(kc == bank_last[a // 512]),
                        )

        # ---- softmax-denominator normalization chains, emitted in slices
        # across later flush points so no in-order engine queue waits on a
        # DMA round-trip hop in flight.
        chains = []

        def flush_chains():
            for ch in list(chains):
                ch.pop(0)()
                if not ch:
                    chains.remove(ch)

        def emit_attention(hl, qh, filler=None):
            con = conA if hl < 2 else conB
            r0 = 64 * (hl % 2)
            outpt = ops.tile([96, 1024], F32, tag="outp", space="PSUM",
                             name=f"outp{hl}_{qh}")
            outp = [outpt[0:65, 0:512], outpt[0:65, 512:1024]]
            emit_attention_bf16(hl, qh, outp, filler)
            box = {}
            if hl == 3 and qh == 1:
                # Last chain: all-on-chip partition spread (no DRAM hops to
                # hide at the very end). The den row is copied to a base-0
                # SBUF tile, DVE 32x32 block transposes put it on
                # partitions, reciprocal runs 32-wide, a second transpose
                # restores a row, and a K=1 PE outer product broadcasts it
                # into PSUM for the con multiply.
                den32 = small.tile([32, 1024], F32, tag="den32", name="den32")
                nc.vector.tensor_copy(den32[0:1, :], outpt[64:65, :])
            stg = small.tile([65, 1024], F32, tag="stg", name=f"stg{hl}_{qh}")
            nc.vector.tensor_copy(stg, outpt[0:65, :])
            if hl == 3 and qh == 1:
                def s2m():
                    tsp = small.tile([32, 1024], F32, tag="tsp", name="tsp")
                    nc.vector.transpose(tsp, den32)
                    rsp = small.tile([32, 1024], F32, tag="rsp", name="rsp")
                    nc.vector.reciprocal(
                        rsp.rearrange("p (b c) -> p b c", c=32)[:, :, 0:1],
                        tsp.rearrange("p (b c) -> p b c", c=32)[:, :, 0:1])
                    r16 = small.tile([32, 1024], BF16, tag="r16", name="r16")
                    nc.vector.tensor_copy(
                        r16.rearrange("p (b c) -> p b c", c=32)[:, :, 0:1],
                        rsp.rearrange("p (b c) -> p b c", c=32)[:, :, 0:1])
                    rT = small.tile([32, 1024], BF16, tag="rT", name="rT")
                    nc.vector.transpose(rT, r16)
                    rec64p = recpool.tile([64, 1024], F32, tag="recp",
                                          space="PSUM", name="recp")
                    for h in range(2):
                        nc.tensor.matmul(
                            rec64p[:, h * 512:(h + 1) * 512],
                            lhsT=ones1,
                            rhs=rT[0:1, h * 512:(h + 1) * 512],
                            start=True, stop=True)
                    box["rec64"] = rec64p

                def s3m():
                    nc.vector.tensor_mul(
                        con[r0:r0 + 64, qh * 1024:(qh + 1) * 1024],
                        stg[0:64, :], box["rec64"])

                # dummy first slice: the real work must land at the final
                # flush, after the end-phase PSUM pools exist
                chains.append([lambda: None, lambda: (s2m(), s3m())])
                return
            # denominator row, partition-spread via DRAM (a (1,1024)
            # single-lane DVE reciprocal costs 6.5us; spread across 128
            # partitions it is ~60ns).
            d_dram = dpool.tile([1, 1024], F32, tag="d_dram", name=f"dd{hl}_{qh}")
            nc.sync.dma_start(out=d_dram, in_=stg[64:65, :])
            spread = small.tile([128, 8], F32, tag="spread", name=f"sp{hl}_{qh}")
            nc.sync.dma_start(
                out=spread,
                in_=d_dram.rearrange("a (p i) -> p a i", p=128)[:, 0, :],
            )

            def s2():
                rspread = small.tile([128, 8], F32, tag="rspread",
                                     name=f"rs{hl}_{qh}")
                nc.vector.reciprocal(rspread, spread)
                # bf16 for the respread round trip: halves the broadcast
                # read (the chain's longest hop); |rec| rel err ~2^-9 is
                # well inside the output tolerance.
                rs16 = small.tile([128, 8], BF16, tag="rs16",
                                  name=f"rs16{hl}_{qh}")
                nc.vector.tensor_copy(rs16, rspread)
                r_dram = dpool.tile([1, 1024], BF16, tag="r_dram",
                                    name=f"rd{hl}_{qh}")
                nc.sync.dma_start(
                    out=r_dram.rearrange("a (p i) -> p a i", p=128)[:, 0, :],
                    in_=rs16,
                )
                rec64 = small.tile([64, 1024], BF16, tag="rec64",
                                   name=f"r64{hl}_{qh}")
                rec_bcast = bass.AP(
                    tensor=r_dram.tensor, offset=r_dram.offset,
                    ap=[[0, 64]] + [list(d) for d in r_dram.ap[1:]],
                )
                nc.sync.dma_start(out=rec64, in_=rec_bcast)
                box["rec64"] = rec64

            def s3():
                # columns stay chunk-major permuted end-to-end: contiguous
                # multiply; the host un-permutes the final output columns.
                nc.vector.tensor_mul(
                    con[r0:r0 + 64, qh * 1024:(qh + 1) * 1024],
                    stg[0:64, :], box["rec64"])

            chains.append([s2, s3])

        # ---- output projection helpers: partialT[j, s] = woutT_s.T @
        # [conA; conB]. The 16 sb=0,1 tiles (which need only the qh=0
        # halves of con) are interleaved one-per-group into the last two
        # attention blocks: their matmuls have no exp dependence, so they
        # absorb the ACT engine's per-group deficit that otherwise stalls
        # the score pipeline, and they shrink the end phase.
        def mmA(ps, sb, jc, stop=False):
            nc.tensor.matmul(
                ps, lhsT=wout_sb[:, 0, jc * 128:(jc + 1) * 128],
                rhs=conA[:, sb * 512:(sb + 1) * 512], start=True, stop=stop)

        def mmB(ps, sb, jc):
            nc.tensor.matmul(
                ps, lhsT=wout_sb[:, 1, jc * 128:(jc + 1) * 128],
                rhs=conB[:, sb * 512:(sb + 1) * 512], start=False, stop=True)

        def finish_pair(sb, m, stage_tiles, engines):
            sts = stpool.tile([128, 2, 512], BF16, tag="st")
            for u in range(2):
                if engines[u] == 's':
                    nc.scalar.copy(sts[:, u, :], stage_tiles[u])
                elif engines[u] == 'g':
                    nc.gpsimd.tensor_copy(sts[:, u, :], stage_tiles[u])
                else:
                    nc.vector.tensor_copy(sts[:, u, :], stage_tiles[u])
            r = (sb * 8 + 2 * m) * 128
            dst = outT[r:r + 256, :].rearrange("(two p) s -> p two s", two=2)
            return sts, dst

        # interleaved sb=0,1 out-proj tiles, one per attention group of the
        # last two blocks; staged on Vector, written on Sync
        sb01_tiles = [(sb, m, u) for sb in (0, 1) for m in range(4)
                      for u in range(2)]
        fstate = {'i': 0, 'sts': None}

        def filler():
            if fstate['i'] >= len(sb01_tiles):
                return
            sb, m, u = sb01_tiles[fstate['i']]
            if u == 0:
                fstate['sts'] = stpool.tile([128, 2, 512], BF16, tag="st",
                                            name=f"st{fstate['i']}")
            ps = opearly.tile([128, 512], F32, tag="ope",
                              name=f"ope{fstate['i']}")
            mmA(ps, sb, 2 * m + u)
            mmB(ps, sb, 2 * m + u)
            nc.vector.tensor_copy(fstate['sts'][:, u, :], ps)
            if u == 1:
                r = (sb * 8 + 2 * m) * 128
                dst = outT[r:r + 256, :].rearrange(
                    "(two p) s -> p two s", two=2)
                nc.sync.dma_start(out=dst, in_=fstate['sts'])
            fstate['i'] += 1

        for hl in range(HPC):
            for qh in range(2):
                emit_attention(hl, qh)
                flush_chains()
        qh0_ctx.close()
        attn_ctx.close()
        oproj = ctx.enter_context(tc.tile_pool(name="oproj", bufs=6, space="PSUM"))
        recpool = ctx.enter_context(
            tc.tile_pool(name="recpool", bufs=1, space="PSUM"))

        # the last chain's s3 is pure Vector work whose DMA inputs are
        # already in flight — emit it first so it rides an empty queue
        while chains:
            flush_chains()
        for sb in (0, 1):
            for m in range(4):
                pss = []
                for u in range(2):
                    ps = oproj.tile([128, 512], F32, tag="op")
                    mmA(ps, sb, 2 * m + u)
                    mmB(ps, sb, 2 * m + u)
                    pss.append(ps)
                sts, dst = finish_pair(sb, m, pss, 'sv')
                nc.sync.dma_start(out=dst, in_=sts)
        pss2 = []
        for m in range(3):
            for u in range(2):
                ps = oproj.tile([128, 512], F32, tag="op")
                mmA(ps, 2, 2 * m + u)
                pss2.append(ps)
        for m in range(3):
            for u in range(2):
                mmB(pss2[2 * m + u], 2, 2 * m + u)
            sts, dst = finish_pair(2, m, pss2[2 * m:2 * m + 2], 'sv')
            nc.sync.dma_start(out=dst, in_=sts)
        for m in range(3, 4):
            pss = []
            for u in range(2):
                ps = oproj.tile([128, 512], F32, tag="op")
                mmA(ps, 2, 2 * m + u)
                mmB(ps, 2, 2 * m + u)
                pss.append(ps)
            sts, dst = finish_pair(2, m, pss, 'sv')
            nc.sync.dma_start(out=dst, in_=sts)
        for m in range(4):
            pss = []
            for u in range(2):
                ps = oproj.tile([128, 512], F32, tag="op")
                mmA(ps, 3, 2 * m + u)
                mmB(ps, 3, 2 * m + u)
                pss.append(ps)
            sts, dst = finish_pair(3, m, pss, 'sv')
            nc.sync.dma_start(out=dst, in_=sts)
    nc.compile()
    return nc


def make_in_maps(x, Wqkv, bqkv, Wout, mm_dt=BF16):
    np_dt = mybir.dt.np(mm_dt)
    x = np.asarray(x, np.float32)
    xT = np.ascontiguousarray(x.transpose(0, 2, 1)).astype(np_dt)  # (2,1024,2048)
    WqkvT = np.asarray(Wqkv, np.float32).T.astype(np_dt)
    WoutT = np.asarray(Wout, np.float32).T.astype(np_dt)
    # wq[(t*8+ec)*128+p, c] = WqkvT[ec*128+p, t*1024+c]
    wqh = np.ascontiguousarray(
        WqkvT.reshape(8, 128, 3, 1024).transpose(2, 0, 1, 3).reshape(3072, 1024))
    # bcol[p, t*8+c] = bqkv[t*1024 + c*128 + p]
    bcolh = np.ascontiguousarray(
        np.asarray(bqkv, np.float32).reshape(3, 8, 128).transpose(2, 0, 1)
        .reshape(128, 24))
    # additive mask for the diagonal 128-chunk: key partitions AND query
    # columns both in within-chunk scatter order (idx=8*jb+s8 -> 16*s8+jb).
    p = np.arange(128)
    pos = 16 * (p % 8) + p // 8
    allowed = pos[:, None] <= pos[None, :]
    maskp = np.where(allowed, 0.0, -1e9).astype(np.float32)
    in_maps = []
    for c in range(8):
        b, qd = divmod(c, 4)
        xc = xT[b][:, qd * SL:(qd + 1) * SL]  # [1024, 512]
        # xq2[p, ec*512 + hl*128 + s] = xc[ec*128+p, hl*128+s]
        xq2h = np.ascontiguousarray(
            xc.reshape(8, 128, 512).transpose(1, 0, 2).reshape(128, 4096))
        wo = WoutT[qd * 256:(qd + 1) * 256, :]  # [256, 1024]
        woq = np.ascontiguousarray(
            wo.reshape(2, 128, E).transpose(1, 0, 2).reshape(128, 2 * E))
        in_maps.append({
            "xq2": xq2h,
            "wq": wqh,
            "bcol": bcolh,
            "woutq": woq,
            "maskp": maskp,
        })
    return in_maps


_NC_CACHE = {}


def get_program(mm_dt=BF16):
    key = ("v2", str(mm_dt))
    if key not in _NC_CACHE:
        _NC_CACHE[key] = build_program(mm_dt)
    return _NC_CACHE[key]


def assemble(results, bout):
    bout = np.asarray(bout, np.float32)
    out = np.zeros((B, S, E), np.float32)
    for c in range(8):
        b = c // 4
        # tile-major [sb, jc, j, permuted scol] -> un-permute columns
        # (stored c4*128 + jb*8 + s8 <-> true c4*128 + 16*s8 + jb) ->
        # [S, E]; partial sum over the 4 head-quads.
        pt = results[c]["partialT"].reshape(4, 8, 128, 4, 16, 8)
        pt = pt.astype(np.float32)
        out[b] += pt.transpose(0, 3, 5, 4, 1, 2).reshape(S, E)
    out += bout
    return out


def kernel(x, Wqkv, bqkv, Wout, bout, mm_dt=BF16, trace=False):
    nc = get_program(mm_dt)
    in_maps = make_in_maps(x, Wqkv, bqkv, Wout, mm_dt)
    res = run_bass_kernel_spmd(nc, in_maps, list(range(8)), trace=trace)
    out = assemble(res.results, bout)
    if trace:
        kernel.last_result = res
    return out


# revision 40
# speedup vs baseline: 1.1480x; 1.0148x over previous
"""Trainium2 Bass kernel for nn_MultiHeadAttention_67018669687091.

Problem: MHA with B=2, S=2048, E=1024, H=16, D=64, causal, fp32.
The reference reshapes (B,S,E)->(B,H,S,D) WITHOUT transpose, so head h of
batch b is the contiguous 128-row x-block rows [h*128,(h+1)*128) viewed as a
(2048, 64) pseudo-sequence: position 16*s+j <- (row s, channel 64j+d).

Sharding: 8 cores; core c owns batch b=c//4 and head-quad qd=c%4 (4 heads).
No collectives: each core writes a row-parallel partial of the output
projection; the host sums the 4 partials per batch and adds bout.

Structure (per core; PE-column-bound, ~0.44 ns per 512-wide matmul column):
 - QKV projection computed DIRECTLY in transposed form: out[e_out, s] =
   sum_e_in WqkvT[e_in, e_out] xT[e_in, s], with all 4 heads' s-ranges
   batched in the matmul free dim (N=512). No PE transposes for Q/K/V.
   V is projected first; each tensor runs as 4 quarter-phases of 2 PSUM
   accumulation chains so the bias-folding drains pipeline mid-phase
   instead of tailing into attention.
 - PSUM drains scatter (e_out-chunk, s) -> (d, permuted pseudo-col) into
   QTs/KTs/VTs; the qkv bias is folded in via per-partition-scalar add
   (DVE) / Identity-activation bias (ACT), alternating engines.
 - Vc (per-chunk V with the ones denominator column) is built by a SINGLE
   SBUF->SBUF XBAR DMA transpose of VTs (padded to 80 partitions); the
   ones row rides through the transpose. Zero PE cost.
 - Per-head causal attention in the chunk-major permuted column domain
   (col = 128*kc + 8*jb + s8 <-> pos 16*(8*kc+s8)+jb). Causal mask applied
   ADDITIVELY (-1e9) to scores in PSUM before the exp, off the exp->PV
   edge. Key chunks whose query lengths sum to 1024 share one St tile and
   ONE exp (the ACT engine is the co-bottleneck of the attention phase).
 - Softmax denominator via the augmented ones-row of V. For 7 of 8
   (head, half) chains the reciprocal is partition-spread via a DRAM
   round trip, emitted in deferred slices across later flush points so
   the hops hide under later attention blocks. The LAST chain stays
   on-chip: DVE 32x32 block transposes spread the den row, reciprocal
   runs 32-wide, and a K=1 PE outer product against a ones row
   broadcasts it into PSUM — no DRAM latency at the very end. The con
   write is a CONTIGUOUS DVE multiply (columns stay permuted end to end;
   the host un-permutes).
 - Output projection: the sb=2 tiles' conA-half matmuls are emitted ahead
   (no dependence on the last chain); staging copies alternate ACT/DVE;
   paired [128,2,512] output DMAs ride the Sync queue.
"""
import numpy as np
from contextlib import ExitStack

import concourse.bass as bass
import concourse.bacc as bacc
import concourse.mybir as mybir
import concourse.tile as tile
from concourse.bass_utils import run_bass_kernel_spmd

E = 1024
H = 16
D = 64
B = 2
S = 2048
HPC = 4          # heads per core
SL = HPC * 128   # x columns per core (512)

F32 = mybir.dt.float32
F32R = mybir.dt.float32r
BF16 = mybir.dt.bfloat16
EXP = mybir.ActivationFunctionType.Exp
IDENT = mybir.ActivationFunctionType.Identity


def _pieces(lo, hi, bank=512):
    """Split [lo, hi) at multiples of `bank` (PSUM bank boundaries)."""
    out = []
    while lo < hi:
        nxt = min(hi, (lo // bank + 1) * bank)
        out.append((lo, nxt))
        lo = nxt
    return out


def build_program(mm_dt=BF16):
    """One SPMD program; per-core data comes via in_maps."""
    assert mm_dt == BF16, "v3 kernel requires a 2-byte dtype (XBAR transpose)"
    nc = bacc.Bacc("TRN2", target_bir_lowering=False)
    DT = mm_dt
    QKVDT = DT if DT == BF16 else F32

    # Host-prepacked inputs (every load a clean 2D DMA):
    #   xq2[p, ec*512 + hl*128 + s] = x[b].T[ec*128+p, qd*512 + hl*128+s]
    #   wq[(t*8+ec)*128+p, c]       = Wqkv.T[ec*128+p, t*1024+c]
    #   bcol[p, t*8+c]              = bqkv[t*1024 + c*128 + p]
    #   woutq[p, hf*E+j]            = Wout.T[qd*256+hf*128+p, j]
    xq2 = nc.dram_tensor("xq2", [128, 8 * 512], DT, kind="ExternalInput").ap()
    wq = nc.dram_tensor("wq", [24 * 128, 1024], DT, kind="ExternalInput").ap()
    bcol = nc.dram_tensor("bcol", [128, 24], F32, kind="ExternalInput").ap()
    woutq = nc.dram_tensor("woutq", [128, 2 * E], DT, kind="ExternalInput").ap()
    maskd = nc.dram_tensor("maskp", [128, 128], F32, kind="ExternalInput").ap()
    # tile-major output: row block (sb*8+jc) holds the [128, 512] tile for
    # out channels jc*128.. and (permuted) positions sb*512.. — every DMA
    # write lands fully contiguous in DRAM; the host assembles/unpermutes.
    outT = nc.dram_tensor("partialT", [32 * 128, 512], BF16,
                          kind="ExternalOutput").ap()

    with tile.TileContext(nc) as tc, ExitStack() as ctx:
        singles = ctx.enter_context(tc.tile_pool(name="singles", bufs=1))
        wpool = ctx.enter_context(tc.tile_pool(name="wpool", bufs=2))
        ppool = ctx.enter_context(tc.tile_pool(name="ppool", bufs=4))
        cpool = ctx.enter_context(tc.tile_pool(name="cpool", bufs=1))
        stpool = ctx.enter_context(tc.tile_pool(name="stpool", bufs=4))
        small = ctx.enter_context(tc.tile_pool(name="small", bufs=3))
        dpool = ctx.enter_context(tc.tile_pool(name="dpool", bufs=2, space="DRAM"))

        # Additive causal mask (0 / -1e9, f32) for the diagonal 128-chunk,
        # host-computed for the permuted key/query order.
        mask01 = singles.tile([128, 128], F32, tag="mask01")

        # Persistent transposed-domain tensors. Head hl occupies columns
        # [hl*2048, (hl+1)*2048); rows 0:64 = d, rows 64:128 zero-padded so
        # score matmuls run K=128 (K=64 serializes LDWEIGHTS). VTs row 64 is
        # the all-ones denominator row; VTs is padded to 80 partitions
        # (XBAR_TILE_SRC_ROWS=16) so ONE SBUF->SBUF DMA transpose produces
        # Vcs[p, chunk, r] = VTs[r, chunk*128+p] — per-chunk V with the ones
        # column riding through at r=64 (cols 65:80 are unread junk).
        QTs = singles.tile([128, HPC * S], DT, tag="QTs")
        KTs = singles.tile([128, HPC * S], DT, tag="KTs")
        VTs = singles.tile([80, HPC * S], QKVDT, tag="VTs")
        Vcs = singles.tile([128, 64, 80], DT, tag="Vcs")

        conA = cpool.tile([128, S], DT, tag="conA")
        conB = cpool.tile([128, S], DT, tag="conB")

        # Startup loads. x chunks + bias + wout issue from the Scalar HWDGE
        # queue, weights (V tensor first) from Sync — the two issuers run in
        # parallel; the first matmul needs only xe[0] + the first w chunk.
        # first two x chunks load individually (small first transfer -> the
        # first matmul starts sooner); the rest in pairs (fewer issues)
        xaps = {}
        for ec in range(2):
            xe = singles.tile([128, 512], DT, tag=f"xe{ec}", name=f"xe{ec}")
            nc.scalar.dma_start(out=xe, in_=xq2[:, ec * 512:(ec + 1) * 512])
            xaps[ec] = xe
        for u in range(1, 4):
            xp = singles.tile([128, 2, 512], DT, tag=f"xp{u}", name=f"xp{u}")
            nc.scalar.dma_start(
                out=xp,
                in_=xq2[:, u * 1024:(u + 1) * 1024].rearrange(
                    "p (two s) -> p two s", two=2))
            xaps[2 * u] = xp[:, 0, :]
            xaps[2 * u + 1] = xp[:, 1, :]
        bcol_sb = singles.tile([128, 24], F32, tag="bcol")
        nc.scalar.dma_start(out=bcol_sb, in_=bcol)
        wout_sb = singles.tile([128, 2, E], DT, tag="wout")

        # zero pads / ones row (needed only from attention onward)
        nc.gpsimd.memset(QTs[64:128, :], 0.0)
        nc.gpsimd.memset(KTs[64:128, :], 0.0)
        nc.gpsimd.memset(VTs[64:65, :], 1.0)
        ones1 = singles.tile([1, 64], DT, tag="ones1")
        nc.gpsimd.memset(ones1, 1.0)

        DEST = {0: QTs, 1: KTs, 2: VTs}

        def emit_drain(t, c, ps):
            # ps[64*jh+d', hl*128 + 8*kc + s8] -> DEST[t][d', permuted col]
            # for j = 2c+jh; bias bqkv[t*1024 + c*128 + p] folded in.
            dest = DEST[t]
            dst5 = dest.rearrange(
                "d (hl kc jb s8) -> d hl kc jb s8", hl=HPC, kc=16, jb=16)
            for jh in range(2):
                src = ps[64 * jh:64 * jh + 64, :].rearrange(
                    "d (hl kc s8) -> d hl kc s8", hl=HPC, kc=16)
                dst = dst5[0:64, :, :, 2 * c + jh, :]
                bias_ap = bcol_sb[64 * jh:64 * jh + 64, t * 8 + c:t * 8 + c + 1]
                if jh == 0:
                    nc.vector.tensor_scalar_add(out=dst, in0=src, scalar1=bias_ap)
                else:
                    nc.scalar.activation(dst, src, IDENT, bias=bias_ap)

        # ---- QKV projection, transposed form. t order (2,0,1): V first so
        # its drains complete long before the Vc transpose needs them.
        # Each tensor runs as 4 quarter-phases of 2 accumulation chains so
        # drains pipeline mid-phase instead of tailing into attention.
        proj_ctx = ExitStack()
        pp = proj_ctx.enter_context(tc.tile_pool(name="pp", bufs=8, space="PSUM"))
        for ti, t in enumerate((2, 0, 1)):
            waps = {}
            for ec in range(2):
                wtc = wpool.tile([128, 1024], DT, tag=f"wt{ec}",
                                 name=f"wt{t}_{ec}")
                r = (t * 8 + ec) * 128
                nc.sync.dma_start(out=wtc, in_=wq[r:r + 128, :])
                waps[ec] = wtc
            for u in range(1, 4):
                wtc = wpool.tile([128, 2, 1024], DT, tag=f"wp{u}",
                                 name=f"wp{t}_{u}")
                r = (t * 8 + 2 * u) * 128
                nc.sync.dma_start(
                    out=wtc,
                    in_=wq[r:r + 256, :].rearrange("(two p) c -> p two c", two=2))
                waps[2 * u] = wtc[:, 0, :]
                waps[2 * u + 1] = wtc[:, 1, :]
            if ti == 0:
                nc.sync.dma_start(out=mask01, in_=maskd)
            for q in range(4):
                cs = (2 * q, 2 * q + 1)
                pss = {c: pp.tile([128, 512], F32, tag="pp", name=f"pp{t}_{c}")
                       for c in cs}
                for ec in range(8):
                    for c in cs:
                        nc.tensor.matmul(
                            pss[c],
                            lhsT=waps[ec][:, c * 128:(c + 1) * 128],
                            rhs=xaps[ec],
                            start=(ec == 0), stop=(ec == 7),
                        )
                for c in cs:
                    emit_drain(t, c, pss[c])
            if ti == 0:
                # Vc via ONE XBAR DMA transpose (scalar HWDGE queue, idle
                # here); completes mid-projection, long before the first PV.
                nc.scalar.dma_start_transpose(out=Vcs, in_=VTs)
        nc.scalar.dma_start(
            out=wout_sb, in_=woutq.rearrange("p (hf j) -> p hf j", hf=2))
        proj_ctx.close()

        attn_ctx = ExitStack()
        ops = attn_ctx.enter_context(tc.tile_pool(name="ops", bufs=1, space="PSUM"))
        qh0_ctx = ExitStack()
        spool = {}
        spool['p'] = qh0_ctx.enter_context(
            tc.tile_pool(name="spsA", bufs=3, space="PSUM"))

        def emit_attention_bf16(hl, qh, outp, filler=None):
            # Key chunks whose query lengths sum to 1024 share one St tile
            # and ONE exp (the causal staircase pairs up exactly).
            q0 = hl * S
            items = []
            for kc in range(8 * (qh + 1)):
                qstart = max(kc * 128, qh * 1024)
                items.append((kc, qstart, (qh + 1) * 1024 - qstart))
            full = [[it] for it in items if it[2] >= 1024]
            rest = sorted((it for it in items if it[2] < 1024),
                          key=lambda it: -it[2])
            groups = list(full)
            i, j = 0, len(rest) - 1
            while i <= j:
                if i < j and rest[i][2] + rest[j][2] <= 1024:
                    groups.append([rest[i], rest[j]])
                    i, j = i + 1, j - 1
                else:
                    groups.append([rest[i]])
                    i += 1
            groups.sort(key=lambda g: min(it[0] for it in g))
            # per-PSUM-bank last writer under the actual emission order
            bank_last = {}
            for g in groups:
                for (kc, qstart, qlen) in g:
                    rel = qstart - qh * 1024
                    for (a, b) in _pieces(rel, rel + qlen):
                        bank_last[a // 512] = kc
            for g in groups:
                if filler is not None:
                    filler()
                St = spool['p'].tile([128, 1024], F32, tag="S", space="PSUM",
                                     name=f"St{hl}_{qh}_{g[0][0]}")
                off, offs = 0, []
                for (kc, qstart, qlen) in g:
                    for (a, b) in _pieces(off, off + qlen):
                        nc.tensor.matmul(
                            St[:, a:b],
                            lhsT=KTs[:, q0 + kc * 128: q0 + (kc + 1) * 128],
                            rhs=QTs[:, q0 + qstart + a - off: q0 + qstart + b - off],
                            start=True, stop=True,
                        )
                    if kc * 128 == qstart:
                        # additive -1e9 on the diagonal chunk, pre-exp
                        nc.vector.tensor_add(
                            St[:, off:off + 128], St[:, off:off + 128], mask01)
                    offs.append(off)
                    off += qlen
                P = ppool.tile([128, 1024], DT, tag="P",
                               name=f"P{hl}_{qh}_{g[0][0]}")
                nc.scalar.activation(P[:, 0:off], St[:, 0:off], EXP, scale=0.125)
                for (kc, qstart, qlen), o in zip(g, offs):
                    rel = qstart - qh * 1024
                    for (a, b) in _pieces(rel, rel + qlen):
                        nc.tensor.matmul(
                            outp[a // 512][:, a % 512:a % 512 + b - a],
                            lhsT=Vcs[:, hl * 16 + kc, 0:65],
                            rhs=P[:, o + a - rel: o + b - rel],
                            start=(kc == 0), stop=(kc == bank_last[a // 512]),
                        )

        # ---- softmax-denominator normalization chains, emitted in slices
        # across later flush points so no in-order engine queue waits on a
        # DMA round-trip hop in flight.
        chains = []

        def flush_chains():
            for ch in list(chains):
                ch.pop(0)()
                if not ch:
                    chains.remove(ch)

        def emit_attention(hl, qh, filler=None):
            con = conA if hl < 2 else conB
            r0 = 64 * (hl % 2)
            outpt = ops.tile([96, 1024], F32, tag="outp", space="PSUM",
                             name=f"outp{hl}_{qh}")
            outp = [outpt[0:65, 0:512], outpt[0:65, 512:1024]]
            emit_attention_bf16(hl, qh, outp, filler)
            box = {}
            if hl == 3 and qh == 1:
                # Last chain: all-on-chip partition spread (no DRAM hops to
                # hide at the very end). The den row is copied to a base-0
                # SBUF tile, DVE 32x32 block transposes put it on
                # partitions, reciprocal runs 32-wide, a second transpose
                # restores a row, and a K=1 PE outer product broadcasts it
                # into PSUM for the con multiply.
                den32 = small.tile([32, 1024], F32, tag="den32", name="den32")
                nc.vector.tensor_copy(den32[0:1, :], outpt[64:65, :])
            stg = small.tile([65, 1024], F32, tag="stg", name=f"stg{hl}_{qh}")
            nc.vector.tensor_copy(stg, outpt[0:65, :])
            if hl == 3 and qh == 1:
                def s2m():
                    tsp = small.tile([32, 1024], F32, tag="tsp", name="tsp")
                    nc.vector.transpose(tsp, den32)
                    rsp = small.tile([32, 1024], F32, tag="rsp", name="rsp")
                    nc.vector.reciprocal(
                        rsp.rearrange("p (b c) -> p b c", c=32)[:, :, 0:1],
                        tsp.rearrange("p (b c) -> p b c", c=32)[:, :, 0:1])
                    r16 = small.tile([32, 1024], BF16, tag="r16", name="r16")
                    nc.vector.tensor_copy(
                        r16.rearrange("p (b c) -> p b c", c=32)[:, :, 0:1],
                        rsp.rearrange("p (b c) -> p b c", c=32)[:, :, 0:1])
                    rT = small.tile([32, 1024], BF16, tag="rT", name="rT")
                    nc.vector.transpose(rT, r16)
                    rec64p = recpool.tile([64, 1024], F32, tag="recp",
                                          space="PSUM", name="recp")
                    for h in range(2):
                        nc.tensor.matmul(
                            rec64p[:, h * 512:(h + 1) * 512],
                            lhsT=ones1,
                            rhs=rT[0:1, h * 512:(h + 1) * 512],
                            start=True, stop=True)
                    box["rec64"] = rec64p

                def s3m():
                    nc.vector.tensor_mul(
                        con[r0:r0 + 64, qh * 1024:(qh + 1) * 1024],
                        stg[0:64, :], box["rec64"])

                # dummy first slice: the real work must land at the final
                # flush, after the end-phase PSUM pools exist
                chains.append([lambda: None, lambda: (s2m(), s3m())])
                return
            # denominator row, partition-spread via DRAM (a (1,1024)
            # single-lane DVE reciprocal costs 6.5us; spread across 128
            # partitions it is ~60ns).
            d_dram = dpool.tile([1, 1024], F32, tag="d_dram", name=f"dd{hl}_{qh}")
            nc.sync.dma_start(out=d_dram, in_=stg[64:65, :])
            spread = small.tile([128, 8], F32, tag="spread", name=f"sp{hl}_{qh}")
            nc.sync.dma_start(
                out=spread,
                in_=d_dram.rearrange("a (p i) -> p a i", p=128)[:, 0, :],
            )

            def s2():
                rspread = small.tile([128, 8], F32, tag="rspread",
                                     name=f"rs{hl}_{qh}")
                nc.vector.reciprocal(rspread, spread)
                # bf16 for the respread round trip: halves the broadcast
                # read (the chain's longest hop); |rec| rel err ~2^-9 is
                # well inside the output tolerance.
                rs16 = small.tile([128, 8], BF16, tag="rs16",
                                  name=f"rs16{hl}_{qh}")
                nc.vector.tensor_copy(rs16, rspread)
                r_dram = dpool.tile([1, 1024], BF16, tag="r_dram",
                                    name=f"rd{hl}_{qh}")
                nc.sync.dma_start(
                    out=r_dram.rearrange("a (p i) -> p a i", p=128)[:, 0, :],
                    in_=rs16,
                )
                rec64 = small.tile([64, 1024], BF16, tag="rec64",
                                   name=f"r64{hl}_{qh}")
                rec_bcast = bass.AP(
                    tensor=r_dram.tensor, offset=r_dram.offset,
                    ap=[[0, 64]] + [list(d) for d in r_dram.ap[1:]],
                )
                nc.sync.dma_start(out=rec64, in_=rec_bcast)
                box["rec64"] = rec64

            def s3():
                # columns stay chunk-major permuted end-to-end: contiguous
                # multiply; the host un-permutes the final output columns.
                nc.vector.tensor_mul(
                    con[r0:r0 + 64, qh * 1024:(qh + 1) * 1024],
                    stg[0:64, :], box["rec64"])

            chains.append([s2, s3])

        # ---- output projection helpers: partialT[j, s] = woutT_s.T @
        # [conA; conB].
        def mmA(ps, sb, jc, stop=False):
            nc.tensor.matmul(
                ps, lhsT=wout_sb[:, 0, jc * 128:(jc + 1) * 128],
                rhs=conA[:, sb * 512:(sb + 1) * 512], start=True, stop=stop)

        def mmB(ps, sb, jc):
            nc.tensor.matmul(
                ps, lhsT=wout_sb[:, 1, jc * 128:(jc + 1) * 128],
                rhs=conB[:, sb * 512:(sb + 1) * 512], start=False, stop=True)

        def finish_pair(sb, m, stage_tiles, engines):
            sts = stpool.tile([128, 2, 512], BF16, tag="st")
            for u in range(2):
                if engines[u] == 's':
                    nc.scalar.copy(sts[:, u, :], stage_tiles[u])
                elif engines[u] == 'g':
                    nc.gpsimd.tensor_copy(sts[:, u, :], stage_tiles[u])
                else:
                    nc.vector.tensor_copy(sts[:, u, :], stage_tiles[u])
            r = (sb * 8 + 2 * m) * 128
            dst = outT[r:r + 256, :].rearrange("(two p) s -> p two s", two=2)
            return sts, dst

        # interleaved sb=0,1 out-proj tiles, one per attention group of the
        # last two blocks; staged on Vector, written on Sync
        for hl in range(HPC):
            for qh in range(2):
                emit_attention(hl, qh)
                flush_chains()
        qh0_ctx.close()
        attn_ctx.close()
        oproj = ctx.enter_context(tc.tile_pool(name="oproj", bufs=6, space="PSUM"))
        recpool = ctx.enter_context(
            tc.tile_pool(name="recpool", bufs=1, space="PSUM"))

        # the last chain's s3 is pure Vector work whose DMA inputs are
        # already in flight — emit it first so it rides an empty queue
        while chains:
            flush_chains()
        for sb in (0, 1):
            for m in range(4):
                pss = []
                for u in range(2):
                    ps = oproj.tile([128, 512], F32, tag="op")
                    mmA(ps, sb, 2 * m + u)
                    mmB(ps, sb, 2 * m + u)
                    pss.append(ps)
                sts, dst = finish_pair(sb, m, pss, 'sv')
                nc.sync.dma_start(out=dst, in_=sts)
        pss2 = []
        for m in range(3):
            for u in range(2):
                ps = oproj.tile([128, 512], F32, tag="op")
                mmA(ps, 2, 2 * m + u)
                pss2.append(ps)
        for m in range(3):
            for u in range(2):
                mmB(pss2[2 * m + u], 2, 2 * m + u)
            sts, dst = finish_pair(2, m, pss2[2 * m:2 * m + 2], 'sv')
            nc.sync.dma_start(out=dst, in_=sts)
        for m in range(3, 4):
            pss = []
            for u in range(2):
                ps = oproj.tile([128, 512], F32, tag="op")
                mmA(ps, 2, 2 * m + u)
                mmB(ps, 2, 2 * m + u)
                pss.append(ps)
            sts, dst = finish_pair(2, m, pss, 'sv')
            nc.sync.dma_start(out=dst, in_=sts)
        for m in range(4):
            pss = []
            for u in range(2):
                ps = oproj.tile([128, 512], F32, tag="op")
                mmA(ps, 3, 2 * m + u)
                mmB(ps, 3, 2 * m + u)
                pss.append(ps)
            sts, dst = finish_pair(3, m, pss, 'sv')
            nc.sync.dma_start(out=dst, in_=sts)
    nc.compile()
    return nc


def make_in_maps(x, Wqkv, bqkv, Wout, mm_dt=BF16):
    np_dt = mybir.dt.np(mm_dt)
    x = np.asarray(x, np.float32)
    xT = np.ascontiguousarray(x.transpose(0, 2, 1)).astype(np_dt)  # (2,1024,2048)
    WqkvT = np.asarray(Wqkv, np.float32).T.astype(np_dt)
    WoutT = np.asarray(Wout, np.float32).T.astype(np_dt)
    # wq[(t*8+ec)*128+p, c] = WqkvT[ec*128+p, t*1024+c]
    wqh = np.ascontiguousarray(
        WqkvT.reshape(8, 128, 3, 1024).transpose(2, 0, 1, 3).reshape(3072, 1024))
    # bcol[p, t*8+c] = bqkv[t*1024 + c*128 + p]
    bcolh = np.ascontiguousarray(
        np.asarray(bqkv, np.float32).reshape(3, 8, 128).transpose(2, 0, 1)
        .reshape(128, 24))
    # additive mask for the diagonal 128-chunk: key partitions AND query
    # columns both in within-chunk scatter order (idx=8*jb+s8 -> 16*s8+jb).
    p = np.arange(128)
    pos = 16 * (p % 8) + p // 8
    allowed = pos[:, None] <= pos[None, :]
    maskp = np.where(allowed, 0.0, -1e9).astype(np.float32)
    in_maps = []
    for c in range(8):
        b, qd = divmod(c, 4)
        xc = xT[b][:, qd * SL:(qd + 1) * SL]  # [1024, 512]
        # xq2[p, ec*512 + hl*128 + s] = xc[ec*128+p, hl*128+s]
        xq2h = np.ascontiguousarray(
            xc.reshape(8, 128, 512).transpose(1, 0, 2).reshape(128, 4096))
        wo = WoutT[qd * 256:(qd + 1) * 256, :]  # [256, 1024]
        woq = np.ascontiguousarray(
            wo.reshape(2, 128, E).transpose(1, 0, 2).reshape(128, 2 * E))
        in_maps.append({
            "xq2": xq2h,
            "wq": wqh,
            "bcol": bcolh,
            "woutq": woq,
            "maskp": maskp,
        })
    return in_maps


_NC_CACHE = {}


def get_program(mm_dt=BF16):
    key = ("v2", str(mm_dt))
    if key not in _NC_CACHE:
        _NC_CACHE[key] = build_program(mm_dt)
    return _NC_CACHE[key]


def assemble(results, bout):
    bout = np.asarray(bout, np.float32)
    out = np.zeros((B, S, E), np.float32)
    for c in range(8):
        b = c // 4
        # tile-major [sb, jc, j, permuted scol] -> un-permute columns
        # (stored c4*128 + jb*8 + s8 <-> true c4*128 + 16*s8 + jb) ->
        # [S, E]; partial sum over the 4 head-quads.
        pt = results[c]["partialT"].reshape(4, 8, 128, 4, 16, 8)
        pt = pt.astype(np.float32)
        out[b] += pt.transpose(0, 3, 5, 4, 1, 2).reshape(S, E)
    out += bout
    return out


def kernel(x, Wqkv, bqkv, Wout, bout, mm_dt=BF16, trace=False):
    nc = get_program(mm_dt)
    in_maps = make_in_maps(x, Wqkv, bqkv, Wout, mm_dt)
    res = run_bass_kernel_spmd(nc, in_maps, list(range(8)), trace=trace)
    out = assemble(res.results, bout)
    if trace:
        kernel.last_result = res
    return out
